# revision 32
# baseline (speedup 1.0000x reference)
"""Trainium2 Bass kernel for a 3-layer ContinuousConv (Open3D-style) point
cloud network + 4-layer FC head.

Strategy (8 NeuronCores, data-parallel over points):
  - 10000 points padded to 10240, sharded 1280/core (1250 real + 30 dummy),
    processed in 10 tiles of 128 points (4 PE row-quadrants x 32).
  - Host precomputes u = (pos[nidx]-pos)*2/EXTENT (masked -> 1e6) and
    remapped neighbor indices; everything else runs on device.
  - All matmul operands are bf16 (fp32 matmuls cost 4 cyc/row on the PE,
    bf16 cost 1); PSUM accumulation stays fp32.
  - Neighbor-count normalization is folded into the x-dimension hat
    weights once (S rows scale by 1/cnt), so conv epilogues are relu+bias.
  - fN gathered by ONE dma_gather per tile (4096 rows of 256 B; x slabs
    stored 128-wide bf16 so rows are 256B-aligned), multi-packet, on 4
    rotating SWDGE queues so consecutive tiles' gathers overlap.
  - Stage 1 (A^T = fN^T @ S per point): 2 matmuls per point (even/odd
    cells) on PE sub-tiles (row grp = point quadrant, col grp pair =
    cell parity). Issue order is q-major (32 same-row-group pairs in a
    row): rapidly alternating row groups with bf16 sub-128 matmuls hard-
    faults the device (HW erratum found by bisection).
  - Stage 2 (out = sum_t A2 @ W[t]): W stationary (bf16), A2 streaming;
    two concurrent 54-step accumulation chains (even ts -> PSUM
    partitions 0:64, odd ts -> 64:128) summed after a PE transpose.
  - Layers 1-2: PE-transpose epilogue writes [128 pts, 64 ch] to DRAM;
    AllGather of the per-core slab between layers. Layer 3 keeps
    [64 ch, 128 pts] and feeds the fused FC head directly.
"""

import os
import numpy as np

KB_NOCC = int(os.environ.get("KB_NOCC", "0"))
KB_L1 = int(os.environ.get("KB_L1", "0"))
KB_TILES = int(os.environ.get("KB_TILES", "0"))
KB_NOGATHER = int(os.environ.get("KB_NOGATHER", "0"))
KB_NOS1 = int(os.environ.get("KB_NOS1", "0"))
KB_DVECOPY = int(os.environ.get("KB_DVECOPY", "0"))
KB_NQ = int(os.environ.get("KB_NQ", "4"))
KB_QMAJOR = int(os.environ.get("KB_QMAJOR", "1"))
KB_G2 = int(os.environ.get("KB_G2", "1"))

# ---------------------------------------------------------------- constants
N = 10000
K = 32
KS = 6
M = 216          # KS^3
HC = 108         # cell pairs
EXTENT = 3.0
EPS = 1e-12
FOUR_OVER_PI = float(4.0 / np.pi)
BIG = 1.0e6

NCORES = 8
PPC = 1250       # real points per core
PT = 128         # points per tile (4 PE quadrants x 32)
NTILES = 10
PPCP = PT * NTILES          # 1280 padded points per core
COLS = NTILES * 32          # 320
NPAD = NCORES * PPCP        # 10240
C = 64           # uniform channel width (padded)
HLF = PPCP // 2  # AllGather half (rows per core per collective)

_CACHE = {}


# ---------------------------------------------------------------- bass build
def _build_program():
    import concourse.bass as bass
    import concourse.tile as tile
    from concourse import mybir, bacc
    from concourse.masks import make_identity
    from contextlib import ExitStack

    f32 = mybir.dt.float32
    bf16 = mybir.dt.bfloat16
    i32 = mybir.dt.int32
    Alu = mybir.AluOpType
    Act = mybir.ActivationFunctionType

    nc = bacc.Bacc("TRN2", target_bir_lowering=False, debug=False,
                   num_devices=NCORES,
                   **({"num_swdge_queues": 4} if KB_G2 else {}))

    # ---- I/O ----
    XW = 128 if KB_G2 else C        # gathered row width (bf16)
    xin = nc.dram_tensor("xin", [NPAD, XW], bf16, kind="ExternalInput")
    nidx_d = nc.dram_tensor("nidx", [128, COLS], i32, kind="ExternalInput")
    if KB_G2:
        i16 = mybir.dt.int16
        idx16_d = nc.dram_tensor("idx16", [128, NTILES * 256], i16,
                                 kind="ExternalInput")
    uin = nc.dram_tensor("uin", [128, 3 * COLS], f32, kind="ExternalInput")
    cnt2_d = nc.dram_tensor("cnt2", [128, COLS], f32, kind="ExternalInput")
    w_d = [nc.dram_tensor(f"w{l}", [128, HC * C], bf16, kind="ExternalInput")
           for l in (1, 2, 3)]
    brow_d = [nc.dram_tensor(f"brow{l}", [1, C], bf16,
                             kind="ExternalInput") for l in (1, 2)]
    bias3_d = nc.dram_tensor("bias3", [64, 1], f32, kind="ExternalInput")
    iota_d = nc.dram_tensor("iota6", [128, 6], f32, kind="ExternalInput")
    wfc_d = [nc.dram_tensor(f"wfc{l}", [64, 64], bf16, kind="ExternalInput")
             for l in (1, 2, 3)]
    wout_d = nc.dram_tensor("wout", [64, 8], bf16, kind="ExternalInput")
    bfc_d = [nc.dram_tensor(f"bfc{l}", [64, 1], f32, kind="ExternalInput")
             for l in (1, 2, 3)]
    bout_d = nc.dram_tensor("bout", [8, 1], f32, kind="ExternalInput")
    outT = nc.dram_tensor("outT", [3, PPCP], f32, kind="ExternalOutput")

    # internal DRAM
    xloc = [nc.dram_tensor(f"xloc{l}", [PPCP, XW], bf16, kind="Internal")
            for l in (1, 2)]
    xfull = [nc.dram_tensor(f"xfull{l}", [NPAD, XW], bf16,
                            addr_space="Shared")
             for l in (1, 2)]

    with tile.TileContext(nc) as tc, ExitStack() as stk:
        # ---------- persistent small constants ----------
        cpool = stk.enter_context(tc.tile_pool(name="const", bufs=1))
        nidx_sb = cpool.tile([128, COLS], i32)
        nc.sync.dma_start(out=nidx_sb[:], in_=nidx_d[:, :])
        if KB_G2:
            idx16_sb = cpool.tile([128, NTILES * 256], mybir.dt.int16,
                                  name="idx16sb")
            nc.sync.dma_start(out=idx16_sb[:], in_=idx16_d[:, :])
        iota_sb = cpool.tile([128, 6], f32)
        nc.sync.dma_start(out=iota_sb[:], in_=iota_d[:, :])
        brow_sb = []
        for l in range(2):
            b = cpool.tile([1, C], bf16, name=f"browsb{l}")
            nc.sync.dma_start(out=b[:], in_=brow_d[l][:, :])
            brow_sb.append(b)
        ones1_sb = cpool.tile([1, PT], bf16)
        nc.vector.memset(ones1_sb[:], 1.0)
        bias3_sb = cpool.tile([64, 1], f32)
        nc.sync.dma_start(out=bias3_sb[:], in_=bias3_d[:, :])
        wfc_sb = []
        for l in range(3):
            w = cpool.tile([64, 64], bf16, name=f"wfcsb{l}")
            nc.sync.dma_start(out=w[:], in_=wfc_d[l][:, :])
            wfc_sb.append(w)
        wout_sb = cpool.tile([64, 8], bf16)
        nc.sync.dma_start(out=wout_sb[:], in_=wout_d[:, :])
        bfc_sb = []
        for l in range(3):
            b = cpool.tile([64, 1], f32, name=f"bfcsb{l}")
            nc.sync.dma_start(out=b[:], in_=bfc_d[l][:, :])
            bfc_sb.append(b)
        bout_sb = cpool.tile([8, 1], f32)
        nc.sync.dma_start(out=bout_sb[:], in_=bout_d[:, :])
        ident_sb = cpool.tile([128, 128], f32)
        make_identity(nc, ident_sb[:])

        # hats: per (j,k) pair the 6-cell 1-D trilinear weights, per dim
        # (bf16; hat0 additionally scaled by 1/neighbor-count)
        hat_sb = [cpool.tile([128, 6 * COLS], f32, name=f"hat{d}")
                  for d in range(3)]
        # hz with cells reordered (h*3 + mz//2) for contiguous ev/od S
        hz2_sb = cpool.tile([128, 6 * COLS], f32, name="hz2")

        # ---------- geometry (ball_to_cube -> grid coords -> hats) ----------
        with tc.tile_pool(name="geo", bufs=1) as geo:
            def gt(tag):
                return geo.tile([128, COLS], f32, name=tag)

            V = nc.vector
            S_ = nc.scalar

            cnt2_sb = geo.tile([128, COLS], f32, name="cnt2sb")
            nc.sync.dma_start(out=cnt2_sb[:], in_=cnt2_d[:, :])

            x = gt("gx"); y = gt("gy"); z = gt("gz")
            nc.sync.dma_start(out=x[:], in_=uin[:, 0:COLS])
            nc.sync.dma_start(out=y[:], in_=uin[:, COLS:2 * COLS])
            nc.sync.dma_start(out=z[:], in_=uin[:, 2 * COLS:3 * COLS])

            u8 = mybir.dt.uint8
            cone_m = geo.tile([128, COLS], u8, name="cone_m")
            xmaj_m = geo.tile([128, COLS], u8, name="xmaj_m")
            den_m = geo.tile([128, COLS], u8, name="den_m")
            ones = gt("ones")
            nc.vector.memset(ones[:], 1.0)

            xx = gt("xx"); yy = gt("yy"); zz = gt("zz")
            V.tensor_mul(xx[:], x[:], x[:])
            V.tensor_mul(yy[:], y[:], y[:])
            V.tensor_mul(zz[:], z[:], z[:])
            rho2 = gt("rho2"); sq = gt("sq")
            V.tensor_add(rho2[:], xx[:], yy[:])
            V.tensor_add(sq[:], rho2[:], zz[:])
            t0 = gt("t0"); norm = gt("norm")
            V.tensor_scalar_max(t0[:], sq[:], EPS)
            S_.activation(norm[:], t0[:], Act.Sqrt)            # norm
            az = gt("az")
            S_.activation(az[:], z[:], Act.Abs)
            den = gt("den")
            V.tensor_add(den[:], norm[:], az[:])
            rden = gt("rden")
            V.reciprocal(rden[:], den[:])
            t1 = gt("t1")
            V.tensor_scalar_mul(t1[:], norm[:], 3.0)
            V.tensor_mul(t1[:], t1[:], rden[:])                # 3n/(n+|z|)
            s1 = gt("s1")
            S_.activation(s1[:], t1[:], Act.Sqrt)
            V.tensor_scalar_max(t0[:], rho2[:], EPS)
            rr = gt("rr")
            V.reciprocal(rr[:], t0[:])
            S_.activation(rr[:], rr[:], Act.Sqrt)              # 1/sqrt(rho2)
            s2 = gt("s2")
            V.tensor_mul(s2[:], norm[:], rr[:])
            cone = gt("cone")
            V.tensor_scalar_mul(cone[:], zz[:], 1.25)
            V.tensor_tensor(cone_m[:], cone[:], rho2[:], op=Alu.is_gt)
            s = gt("s")
            V.select(s[:], cone_m[:], s1[:], s2[:])
            xc = gt("xc"); yc = gt("yc"); zc = gt("zc")
            V.tensor_mul(xc[:], x[:], s[:])
            V.tensor_mul(yc[:], y[:], s[:])
            sgn = gt("sgn")
            S_.activation(sgn[:], z[:], Act.Sign)
            V.tensor_mul(sgn[:], sgn[:], norm[:])              # sign(z)*norm
            t2 = gt("t2")
            V.tensor_scalar_mul(t2[:], z[:], 1.5)
            V.select(zc[:], cone_m[:], sgn[:], t2[:])
            tm = gt("tm")
            V.tensor_scalar(tm[:], sq[:], EPS, None, op0=Alu.is_ge)
            V.tensor_mul(xc[:], xc[:], tm[:])
            V.tensor_mul(yc[:], yc[:], tm[:])
            V.tensor_mul(zc[:], zc[:], tm[:])

            # cylinder -> cube (xy disc)
            V.tensor_mul(xx[:], xc[:], xc[:])
            V.tensor_mul(yy[:], yc[:], yc[:])
            sqxy = gt("sqxy")
            V.tensor_add(sqxy[:], xx[:], yy[:])
            V.tensor_scalar_max(t0[:], sqxy[:], EPS)
            nxy = gt("nxy")
            S_.activation(nxy[:], t0[:], Act.Sqrt)
            axc = gt("axc"); ayc = gt("ayc")
            S_.activation(axc[:], xc[:], Act.Abs)
            S_.activation(ayc[:], yc[:], Act.Abs)
            V.tensor_tensor(xmaj_m[:], ayc[:], axc[:], op=Alu.is_le)
            sgx = gt("sgx"); sgy = gt("sgy")
            S_.activation(sgx[:], xc[:], Act.Sign)
            S_.activation(sgy[:], yc[:], Act.Sign)
            tx = gt("txv"); ty = gt("tyv")
            V.tensor_mul(tx[:], sgx[:], nxy[:])
            V.tensor_mul(ty[:], sgy[:], nxy[:])
            # safe denominators
            V.tensor_scalar(den_m[:], axc[:], EPS, None, op0=Alu.is_lt)
            xd = gt("xd")
            V.select(xd[:], den_m[:], ones[:], xc[:])
            V.tensor_scalar(den_m[:], ayc[:], EPS, None, op0=Alu.is_lt)
            yd = gt("yd")
            V.select(yd[:], den_m[:], ones[:], yc[:])
            V.reciprocal(t1[:], yd[:])
            V.tensor_mul(t1[:], xc[:], t1[:])
            V.tensor_scalar(t1[:], t1[:], 1.0, -1.0, op0=Alu.min,
                            op1=Alu.max)           # clamp unused branch
            at1 = gt("at1")
            S_.activation(at1[:], t1[:], Act.Arctan)
            V.reciprocal(t2[:], xd[:])
            V.tensor_mul(t2[:], yc[:], t2[:])
            V.tensor_scalar(t2[:], t2[:], 1.0, -1.0, op0=Alu.min,
                            op1=Alu.max)
            at2 = gt("at2")
            S_.activation(at2[:], t2[:], Act.Arctan)
            # xq
            V.tensor_mul(t1[:], ty[:], at1[:])
            V.tensor_scalar_mul(t1[:], t1[:], FOUR_OVER_PI)
            xq = gt("xq")
            V.select(xq[:], xmaj_m[:], tx[:], t1[:])
            # yq
            V.tensor_mul(t2[:], tx[:], at2[:])
            V.tensor_scalar_mul(t2[:], t2[:], FOUR_OVER_PI)
            yq = gt("yq")
            V.select(yq[:], xmaj_m[:], t2[:], ty[:])
            V.tensor_scalar(tm[:], sqxy[:], EPS, None, op0=Alu.is_ge)
            V.tensor_mul(xq[:], xq[:], tm[:])
            V.tensor_mul(yq[:], yq[:], tm[:])

            # grid coords (align_corners): (c+1)*2.5
            coords = []
            for src, tag in ((xq, "ccx"), (yq, "ccy"), (zc, "ccz")):
                cd = gt(tag)
                V.tensor_scalar(cd[:], src[:], 1.0, 2.5, op0=Alu.add,
                                op1=Alu.mult)
                coords.append(cd)

            # hats: w[p, col*6+m] = relu(1 - |iota6[m] - coord[p,col]|)
            iap = iota_sb[:]
            hraw = geo.tile([128, 6 * COLS], f32, name="hraw")
            for d in range(3):
                cap = coords[d][:]
                hat = hat_sb[d]
                io_b = bass.AP(iap.tensor, iap.offset,
                               [iap.ap[0], [0, COLS], [1, 6]])
                cd_b = bass.AP(cap.tensor, cap.offset,
                               [cap.ap[0], [1, COLS], [0, 6]])
                if d == 0:
                    # relu(1-|d|) in fp32, then scale by 1/cnt
                    V.tensor_tensor(hraw[:], io_b, cd_b, op=Alu.subtract)
                    S_.activation(hraw[:], hraw[:], Act.Abs)
                    S_.activation(hraw[:], hraw[:], Act.Relu,
                                  bias=1.0, scale=-1.0)
                    c2 = cnt2_sb[:]
                    c2_b = bass.AP(c2.tensor, c2.offset,
                                   [c2.ap[0], [1, COLS], [0, 6]])
                    V.tensor_tensor(hat[:], hraw[:], c2_b, op=Alu.mult)
                else:
                    V.tensor_tensor(hraw[:], io_b, cd_b, op=Alu.subtract)
                    S_.activation(hraw[:], hraw[:], Act.Abs)
                    S_.activation(hat[:], hraw[:], Act.Relu,
                                  bias=1.0, scale=-1.0)       # relu(1-|d|)
            # hz2[p, col*6 + h*3 + u] = hz[p, col*6 + 2u + h]
            hz = hat_sb[2][:]
            hz_r = bass.AP(hz.tensor, hz.offset,
                           [hz.ap[0], [6, COLS], [1, 2], [2, 3]])
            z2 = hz2_sb[:]
            z2_o = bass.AP(z2.tensor, z2.offset,
                           [z2.ap[0], [6, COLS], [3, 2], [1, 3]])
            V.tensor_copy(z2_o, hz_r)

        # ---------- conv layers ----------
        wpool = stk.enter_context(tc.tile_pool(name="wpool", bufs=2))
        fnpool = stk.enter_context(tc.tile_pool(name="fn", bufs=3))
        wyzpool = stk.enter_context(tc.tile_pool(name="wyz", bufs=2))
        spool = stk.enter_context(tc.tile_pool(name="spool", bufs=2))
        apool = stk.enter_context(tc.tile_pool(name="apool", bufs=2))
        xpool = stk.enter_context(tc.tile_pool(name="xpool", bufs=2))
        psA = stk.enter_context(tc.tile_pool(name="psA", bufs=3, space="PSUM"))
        psO = stk.enter_context(tc.tile_pool(name="psO", bufs=2, space="PSUM"))
        psT = stk.enter_context(tc.tile_pool(name="psT", bufs=1, space="PSUM"))
        psF = stk.enter_context(tc.tile_pool(name="psF", bufs=1, space="PSUM"))
        psG = stk.enter_context(tc.tile_pool(name="psG", bufs=1, space="PSUM"))

        def conv_layer(li, xsrc, xdst):
            last = xdst is None
            wsb = wpool.tile([128, HC * C], bf16, tag="W", name=f"wsb{li}")
            nc.sync.dma_start(out=wsb[:], in_=w_d[li][:, :])
            for t in range(KB_TILES or NTILES):
                # gather fN: 32 per-column indirect DMAs (one offset per
                # partition-row is the HW granularity)
                fnb = fnpool.tile([128, 32 * XW], bf16, tag="fnb",
                                  name=f"fnb{li}_{t}")
                if KB_NOGATHER:
                    nc.vector.memset(fnb[:], 0.01)
                elif KB_G2:
                    fap = fnb[:]
                    out3 = bass.AP(fap.tensor, fap.offset,
                                   [fap.ap[0], [XW, 32], [1, XW]])
                    nc.gpsimd.dma_gather(
                        out3, xsrc[:, :],
                        idx16_sb[:, t * 256:(t + 1) * 256],
                        32 * 128, 32 * 128, XW, single_packet=False,
                        queue_num=(t % 4) if KB_G2 else 0)
                else:
                    for b in range(32):
                        cb = t * 32 + b
                        nc.gpsimd.indirect_dma_start(
                            out=fnb[:, b * C:(b + 1) * C], out_offset=None,
                            in_=xsrc[:, :],
                            in_offset=bass.IndirectOffsetOnAxis(
                                ap=nidx_sb[:, cb:cb + 1], axis=0),
                        )
                # S tiles (ev/od cells separately, contiguous per point):
                # st_h[p, b*108 + mx*18 + my*3 + u] , pair id = m//2
                wyz = wyzpool.tile([128, 32 * 36], f32, tag="wyz",
                                   name=f"wyz{li}_{t}")
                hy = hat_sb[1][:]
                z2 = hz2_sb[:]
                wzap = wyz[:]
                for h in range(2):
                    hy_b = bass.AP(hy.tensor, hy.offset + t * 192,
                                   [hy.ap[0], [6, 32], [1, 6], [0, 3]])
                    z2_b = bass.AP(z2.tensor, z2.offset + t * 192 + h * 3,
                                   [z2.ap[0], [6, 32], [0, 6], [1, 3]])
                    wz_o = bass.AP(wzap.tensor, wzap.offset + h * 18,
                                   [wzap.ap[0], [36, 32], [3, 6], [1, 3]])
                    nc.vector.tensor_tensor(wz_o, hy_b, z2_b, op=Alu.mult)
                hx = hat_sb[0][:]
                st_h = []
                for h in range(2):
                    sth = spool.tile([128, 32 * 108], bf16, tag=f"S{h}",
                                     name=f"st{li}_{t}_{h}")
                    hx_b = bass.AP(hx.tensor, hx.offset + t * 192,
                                   [hx.ap[0], [6, 32], [1, 6], [0, 18]])
                    wz_b = bass.AP(wzap.tensor, wzap.offset + h * 18,
                                   [wzap.ap[0], [36, 32], [0, 6], [1, 18]])
                    nc.vector.tensor_tensor(sth[:], hx_b, wz_b, op=Alu.mult)
                    st_h.append(sth)

                # stage 1: per-point A^T, 4 points (quadrants) per PSUM tile
                a2 = apool.tile([128, PT * HC], bf16, tag="A2",
                                name=f"a2_{li}_{t}")
                if KB_NOS1:
                    nc.vector.memset(a2[:], 0.01)
                elif KB_QMAJOR:
                    # q-major: 32 same-row-group matmul pairs in a row;
                    # PSUM groups 4 consecutive b of one q, strided a2 dst
                    a2ap0 = a2[:]
                    for q in range(KB_NQ):
                        for b0 in range(0, 32, 4):
                            ps = psA.tile([128, 4 * HC], f32, tag="psA",
                                          name=f"psA{li}_{t}_{q}_{b0}")
                            for w_ in range(4):
                                b = b0 + w_
                                fsl = fnb[q * 32:(q + 1) * 32,
                                          b * XW:b * XW + C]
                                s_ev = st_h[0][q * 32:(q + 1) * 32,
                                              b * HC:(b + 1) * HC]
                                s_od = st_h[1][q * 32:(q + 1) * 32,
                                              b * HC:(b + 1) * HC]
                                nc.tensor.matmul(
                                    ps[0:64, w_ * HC:(w_ + 1) * HC],
                                    fsl, s_ev, start=True, stop=True,
                                    tile_position=(q * 32, 0))
                                nc.tensor.matmul(
                                    ps[64:128, w_ * HC:(w_ + 1) * HC],
                                    fsl, s_od, start=True, stop=True,
                                    tile_position=(q * 32, 64))
                            dst = bass.AP(
                                a2ap0.tensor,
                                a2ap0.offset + (b0 * 4 + q) * HC,
                                [a2ap0.ap[0], [4 * HC, 4], [1, HC]])
                            if (q + b0) % 8 == 1 or KB_DVECOPY:
                                nc.vector.tensor_copy(dst, ps[:])
                            else:
                                nc.scalar.copy(dst, ps[:])
                if not KB_NOS1 and not KB_QMAJOR:
                  for b in range(32):
                    ps = psA.tile([128, 4 * HC], f32, tag="psA",
                                  name=f"psA{li}_{t}_{b}")
                    for q in range(KB_NQ):
                        fsl = fnb[q * 32:(q + 1) * 32, b * XW:b * XW + C]
                        s_ev = st_h[0][q * 32:(q + 1) * 32,
                                      b * HC:(b + 1) * HC]
                        s_od = st_h[1][q * 32:(q + 1) * 32,
                                      b * HC:(b + 1) * HC]
                        nc.tensor.matmul(ps[0:64, q * HC:(q + 1) * HC],
                                         fsl, s_ev, start=True, stop=True,
                                         tile_position=(q * 32, 0))
                        nc.tensor.matmul(ps[64:128, q * HC:(q + 1) * HC],
                                         fsl, s_od, start=True, stop=True,
                                         tile_position=(q * 32, 64))
                    dst = a2[:, b * 4 * HC:(b + 1) * 4 * HC]
                    if b % 8 == 7 or KB_DVECOPY:
                        nc.vector.tensor_copy(dst, ps[:])
                    else:
                        nc.scalar.copy(dst, ps[:])

                # stage 2: accumulate over 108 cell pairs, W stationary.
                po = psO.tile([128, PT], f32, tag="psO", name=f"psO{li}_{t}")
                a2ap = a2[:]
                if KB_L1 == 3:
                    # debug: skip stage 2; dump a slice of a2
                    xt = xpool.tile([128, C], bf16, tag="xt",
                                    name=f"xt{li}_{t}")
                    nc.vector.tensor_copy(xt[:], a2[:, 0:C])
                    nc.sync.dma_start(out=xdst[t * PT:(t + 1) * PT, 0:C],
                                      in_=xt[:])
                elif KB_L1 == 2 and not last:
                    # debug: single-chain stage 2, no bias MM, no halves-sum
                    for ts_ in range(HC):
                        rhs = bass.AP(a2ap.tensor, a2ap.offset + ts_,
                                      [a2ap.ap[0], [HC, PT]])
                        nc.tensor.matmul(po[0:64, :],
                                         wsb[:, ts_ * C:(ts_ + 1) * C], rhs,
                                         start=(ts_ == 0),
                                         stop=(ts_ == HC - 1))
                    xtt = xpool.tile([64, PT], f32, tag="xtt",
                                     name=f"xtt_{li}_{t}")
                    nc.scalar.activation(xtt[:], po[0:64, :], Act.Relu)
                    pt_ = psT.tile([128, 64], f32, tag="psT",
                                   name=f"psT{li}_{t}")
                    nc.tensor.transpose(pt_[:], xtt[:], ident_sb[0:64, 0:64])
                    xt = xpool.tile([128, C], bf16, tag="xt",
                                    name=f"xt{li}_{t}")
                    nc.vector.tensor_copy(xt[:], pt_[:])
                    nc.sync.dma_start(out=xdst[t * PT:(t + 1) * PT, :],
                                      in_=xt[:])
                elif not last:
                    # two concurrent chains: even ts -> parts 0:64, odd ts
                    # -> parts 64:128 (disjoint PE col groups)
                    for i in range(HC // 2):
                        for h in range(2):
                            ts_ = 2 * i + h
                            rhs = bass.AP(a2ap.tensor, a2ap.offset + ts_,
                                          [a2ap.ap[0], [HC, PT]])
                            nc.tensor.matmul(
                                po[64 * h:64 * (h + 1), :],
                                wsb[:, ts_ * C:(ts_ + 1) * C], rhs,
                                start=(i == 0),
                                stop=(h == 1 and i == HC // 2 - 1))
                    # bias joins the lo chain as bias (x) ones
                    nc.tensor.matmul(po[0:64, :], brow_sb[li][:],
                                     ones1_sb[:], start=False, stop=True)
                    # out = relu(lo + hi), then transpose to [128pt, 64ch]
                    c1 = xpool.tile([64, PT], f32, tag="c1",
                                    name=f"c1_{li}_{t}")
                    nc.scalar.copy(c1[:], po[64:128, :])
                    sm = xpool.tile([64, PT], f32, tag="sm",
                                    name=f"sm_{li}_{t}")
                    nc.vector.tensor_add(sm[:], po[0:64, :], c1[:])
                    xtt = xpool.tile([64, PT], f32, tag="xtt",
                                     name=f"xtt_{li}_{t}")
                    nc.scalar.activation(xtt[:], sm[:], Act.Relu)
                    pt_ = psT.tile([128, 64], f32, tag="psT",
                                   name=f"psT{li}_{t}")
                    nc.tensor.transpose(pt_[:], xtt[:], ident_sb[0:64, 0:64])
                    xt = xpool.tile([128, C], bf16, tag="xt",
                                    name=f"xt{li}_{t}")
                    nc.vector.tensor_copy(xt[:], pt_[:])
                    nc.sync.dma_start(
                        out=xdst[t * PT:(t + 1) * PT, 0:C], in_=xt[:])
                else:
                    # single chain (keeps [64ch, 128pt] for the FC head)
                    for ts_ in range(HC):
                        rhs = bass.AP(a2ap.tensor, a2ap.offset + ts_,
                                      [a2ap.ap[0], [HC, PT]])
                        nc.tensor.matmul(po[0:64, :],
                                         wsb[:, ts_ * C:(ts_ + 1) * C], rhs,
                                         start=(ts_ == 0), stop=(ts_ == HC - 1))
                    h = xpool.tile([64, PT], bf16, tag="h0", name=f"h0_{t}")
                    nc.scalar.activation(h[:], po[0:64, :], Act.Relu,
                                         bias=bias3_sb[:])
                    for l in range(3):
                        pf = psF.tile([64, PT], f32, tag="psF",
                                      name=f"psf{t}_{l}")
                        nc.tensor.matmul(pf[:], wfc_sb[l][:], h[:],
                                         start=True, stop=True)
                        h = xpool.tile([64, PT], bf16, tag=f"h{l + 1}",
                                       name=f"h{l + 1}_{t}")
                        nc.scalar.activation(h[:], pf[:], Act.Relu,
                                             bias=bfc_sb[l][:])
                    pg = psG.tile([8, PT], f32, tag="psG", name=f"psG{t}")
                    nc.tensor.matmul(pg[:], wout_sb[:], h[:],
                                     start=True, stop=True)
                    ot = xpool.tile([8, PT], f32, tag="ot", name=f"ot{t}")
                    nc.vector.tensor_scalar(ot[:], pg[:], bout_sb[:], None,
                                            op0=Alu.add)
                    nc.sync.dma_start(out=outT[:, t * PT:(t + 1) * PT],
                                      in_=ot[0:3, :])

            if not last and not KB_NOCC:
                # half-0 fires once tiles 0-4 stored (overlaps tiles 5-9);
                # only half-1 remains on the layer boundary
                nc.gpsimd.collective_compute(
                    "AllGather", Alu.bypass,
                    replica_groups=[list(range(NCORES))],
                    ins=[xdst[0:HLF, :]],
                    outs=[xfull[li][0:NCORES * HLF, :]],
                )
                nc.gpsimd.collective_compute(
                    "AllGather", Alu.bypass,
                    replica_groups=[list(range(NCORES))],
                    ins=[xdst[HLF:PPCP, :]],
                    outs=[xfull[li][NCORES * HLF:NPAD, :]],
                )

        conv_layer(0, xin, xloc[0])
        if not KB_L1:
            conv_layer(1, xfull[0], xloc[1])
            conv_layer(2, xfull[1], None)
        else:
            zt = xpool.tile([8, PT], f32, tag="zt")
            nc.vector.memset(zt[:], 0.0)
            for t in range(KB_TILES or NTILES):
                nc.sync.dma_start(out=outT[:, t * PT:(t + 1) * PT],
                                  in_=zt[0:3, :])

    nc.compile()
    return nc


# ---------------------------------------------------------------- host prep
def _layout_per_core(V):
    """[PPCP, K] -> [128, COLS] with out[q*32+k, t*32+b] = V[t*128+b*4+q, k]."""
    return (V.reshape(NTILES, 32, 4, K)      # [t, b, q, k]
            .transpose(2, 3, 0, 1)           # [q, k, t, b]
            .reshape(128, COLS))


def _bf16(a):
    import ml_dtypes
    return np.asarray(a, np.float32).astype(ml_dtypes.bfloat16)


def _prep_inputs(feats, pos, neighbor_idx, neighbor_mask,
                 W1, b1, W2, b2, W3, b3,
                 Wfc1, bfc1, Wfc2, bfc2, Wfc3, bfc3, Wout, bout):
    f4 = np.asarray(feats, np.float32)
    pos = np.asarray(pos, np.float32)
    nidx = np.asarray(neighbor_idx, np.int32)
    nmask = np.asarray(neighbor_mask, bool)

    # u (masked -> BIG), cnt_inv
    u = (pos[nidx] - pos[:, None, :]) * np.float32(2.0 / EXTENT)
    u = np.where(nmask[..., None], u, np.float32(BIG)).astype(np.float32)
    cnt = nmask.sum(axis=1)
    cnt_inv = (1.0 / np.maximum(cnt, 1)).astype(np.float32)

    # global index -> padded allgather row (half-major: the two
    # half-AllGathers each concatenate rank chunks contiguously)
    g = nidx.astype(np.int64)
    gc = g // PPC
    gj = g % PPC
    remap = ((gj // HLF) * (NCORES * HLF) + gc * HLF
             + (gj % HLF)).astype(np.int32)

    # xin: [NPAD, XW] bf16
    XW = 128 if KB_G2 else C
    xin = np.zeros((NPAD, XW), np.float32)
    jj = np.arange(PPC)
    for c in range(NCORES):
        rows = (jj // HLF) * (NCORES * HLF) + c * HLF + (jj % HLF)
        xin[rows, :f4.shape[1]] = f4[c * PPC:(c + 1) * PPC]
    xin = _bf16(xin)

    def warr(W, cin, cout):
        Wp = np.zeros((M, C, C), np.float32)
        Wp[:, :cin, :cout] = np.asarray(W, np.float32).reshape(M, cin, cout)
        return _bf16(Wp.reshape(HC, 2, C, C).transpose(1, 2, 0, 3)
                     .reshape(128, HC * C))

    w1 = warr(W1, 4, 64)
    w2 = warr(W2, 64, 64)
    w3 = warr(W3, 64, 32)

    def brow(b, n):
        bp = np.zeros((1, C), np.float32)
        bp[0, :n] = np.asarray(b, np.float32)
        return _bf16(bp)

    brow1, brow2 = brow(b1, 64), brow(b2, 64)
    bias3 = np.zeros((64, 1), np.float32)
    bias3[:32, 0] = np.asarray(b3, np.float32)
    iota6 = np.tile(np.arange(6, dtype=np.float32), (128, 1)).copy()

    wfc1 = np.zeros((64, 64), np.float32)
    wfc1[:32, :] = np.asarray(Wfc1, np.float32)
    wfc2 = np.asarray(Wfc2, np.float32).copy()
    wfc3 = np.zeros((64, 64), np.float32)
    wfc3[:, :32] = np.asarray(Wfc3, np.float32)
    wout = np.zeros((64, 8), np.float32)
    wout[:32, :3] = np.asarray(Wout, np.float32)

    def bcol(b, n, p):
        v = np.zeros((p, 1), np.float32)
        v[:n, 0] = np.asarray(b, np.float32)
        return v

    bfc1c, bfc2c, bfc3c = bcol(bfc1, 64, 64), bcol(bfc2, 64, 64), \
        bcol(bfc3, 32, 64)
    boutc = bcol(bout, 3, 8)

    in_maps = []
    for c in range(NCORES):
        # per-core padded [PPCP, K] views
        uloc = np.full((PPCP, K, 3), BIG, np.float32)
        uloc[:PPC] = u[c * PPC:(c + 1) * PPC]
        nloc = np.zeros((PPCP, K), np.int32)
        nloc[:PPC] = remap[c * PPC:(c + 1) * PPC]
        cloc = np.ones(PPCP, np.float32)
        cloc[:PPC] = cnt_inv[c * PPC:(c + 1) * PPC]

        uin = np.concatenate(
            [_layout_per_core(uloc[:, :, d]) for d in range(3)],
            axis=1).astype(np.float32).copy()
        nidx_dev = _layout_per_core(nloc).astype(np.int32).copy()
        # cnt2[q*32+k, t*32+b] = cnt_inv of point (t,q,b), same for all k
        cnt2 = _layout_per_core(
            np.tile(cloc[:, None], (1, K))).astype(np.float32).copy()

        extra = {}
        if XW == 128:
            # dma_gather wrapped int16 indices: flat slot n = b*128 + p,
            # wrapped at (n%16 + 16*r, n//16) for all replicas r
            nl = _layout_per_core(nloc).astype(np.int64)   # [128, COLS]
            idx16 = np.zeros((128, NTILES * 256), np.int16)
            n = np.arange(4096)
            for t in range(NTILES):
                vals = nl[n % 128, t * 32 + n // 128].astype(np.int16)
                for r in range(8):
                    idx16[(n % 16) + 16 * r, t * 256 + n // 16] = vals
            extra["idx16"] = idx16
        in_maps.append({
            **extra,
            "xin": xin, "nidx": nidx_dev, "uin": uin, "cnt2": cnt2,
            "w1": w1, "w2": w2, "w3": w3,
            "brow1": brow1, "brow2": brow2, "bias3": bias3,
            "iota6": iota6,
            "wfc1": _bf16(wfc1), "wfc2": _bf16(wfc2), "wfc3": _bf16(wfc3),
            "wout": _bf16(wout),
            "bfc1": bfc1c, "bfc2": bfc2c, "bfc3": bfc3c, "bout": boutc,
        })
    return in_maps


def _run(in_maps, trace=False, **kw):
    from concourse.bass_utils import run_bass_kernel_spmd
    if "nc" not in _CACHE:
        _CACHE["nc"] = _build_program()
    nc = _CACHE["nc"]
    res = run_bass_kernel_spmd(nc, in_maps, core_ids=list(range(NCORES)),
                               trace=trace, **kw)
    return res


def kernel(**inputs):
    in_maps = _prep_inputs(**{k: np.asarray(v) for k, v in inputs.items()})
    res = _run(in_maps)
    outs = []
    for c in range(NCORES):
        oc = np.asarray(res.results[c]["outT"], np.float32)   # [3, PPCP]
        outs.append(oc[:, :PPC].T)                            # [PPC, 3]
    return np.concatenate(outs, axis=0).astype(np.float32)
